# revision 9
# baseline (speedup 1.0000x reference)
"""Trainium2 Bass kernel for y = inputs @ weights.T + bias.

Shapes: inputs [8192, 4096] f32, weights [4096, 4096] f32, bias [4096] f32,
output [8192, 4096] f32.

Strategy:
- Data-parallel across 8 NeuronCores: each core computes 1024 rows of the
  output; weights/bias are replicated.
- Host pre-transposes inputs and weights to K-major layout so the device
  kernel needs no on-chip transposes (fp32 has no DMA-transpose path).
- Matmuls run in float32r (TF32-like, full PE rate at moving-dim >= 256,
  ~1.5e-4 rel err at K=4096) accumulating fp32 in PSUM.
- Per core: cache the x-slice KxM [4096,1024] in SBUF (16.8 MB), stream W
  in [128,512] tiles, 8 PSUM banks accumulate the 8 row-blocks over K,
  bias added on DVE during PSUM eviction.
"""

import numpy as np

import concourse.bacc as bacc
import concourse.mybir as mybir
import concourse.tile as tile
from concourse.bass_utils import run_bass_kernel_spmd

N_CORES = 8
N_FULL = 8192  # input rows
K_DIM = 4096  # contraction (in features)
O_DIM = 4096  # out features
M = N_FULL // N_CORES  # rows per core (1024)
P = 128
KO = K_DIM // P  # 32 k-tiles
N_TILE = 512  # moving free dim per matmul (1 PSUM bank of fp32)
N_BLOCKS = O_DIM // N_TILE  # 8
M_BLOCKS = M // P  # 8

_nc_cache = None


def _build():
    nc = bacc.Bacc(target_bir_lowering=False)

    xT = nc.dram_tensor("xT", [K_DIM, M], mybir.dt.float32r, kind="ExternalInput")
    wT = nc.dram_tensor("wT", [K_DIM, O_DIM], mybir.dt.float32r, kind="ExternalInput")
    biasr = nc.dram_tensor("biasr", [P, O_DIM], mybir.dt.float32, kind="ExternalInput")
    y = nc.dram_tensor("y", [M, O_DIM], mybir.dt.float32, kind="ExternalOutput")

    xT3 = xT.ap().rearrange("(ko p) m -> p ko m", p=P)
    wT3 = wT.ap().rearrange("(ko p) n -> p ko n", p=P)
    y3 = y.ap().rearrange("(mb p) n -> p mb n", p=P)

    with tile.TileContext(nc) as tc:
        with (
            tc.tile_pool(name="persist", bufs=1) as persist,
            tc.tile_pool(name="wpool", bufs=10) as wpool,
            tc.tile_pool(name="opool", bufs=10) as opool,
            tc.tile_pool(name="psum", bufs=1, space="PSUM") as psum_pool,
        ):
            # Bias first (tiny, and the first PSUM drain gates on it).
            bias_sb = persist.tile([P, O_DIM], mybir.dt.float32, tag="bias")
            nc.sync.dma_start(bias_sb[:], biasr.ap())

            # x cached in SBUF, one tile per k-slab so matmuls can start as
            # soon as their slab has landed; loads split across two queues.
            x_sb = []
            for ko in range(KO):
                x_t = persist.tile([P, M], mybir.dt.float32r, tag=f"x{ko}")
                xeng = nc.gpsimd if ko % 2 == 0 else nc.scalar
                xeng.dma_start(x_t[:], xT3[:, ko])
                x_sb.append(x_t)

            for nb in range(N_BLOCKS):
                psums = [
                    psum_pool.tile(
                        [P, N_TILE], mybir.dt.float32, tag=f"ps{m}", name=f"ps{m}"
                    )
                    for m in range(M_BLOCKS)
                ]
                for ko in range(KO):
                    w_t = wpool.tile([P, N_TILE], mybir.dt.float32r, tag="w")
                    weng = nc.sync if ko % 2 == 0 else nc.scalar
                    weng.dma_start(
                        w_t[:], wT3[:, ko, nb * N_TILE : (nb + 1) * N_TILE]
                    )
                    for mb in range(M_BLOCKS):
                        nc.tensor.matmul(
                            psums[mb][:],
                            x_sb[ko][:, mb * P : (mb + 1) * P],
                            w_t[:],
                            start=(ko == 0),
                            stop=(ko == KO - 1),
                        )
                for mb in range(M_BLOCKS):
                    o_t = opool.tile([P, N_TILE], mybir.dt.float32, tag="o")
                    nc.any.tensor_add(
                        o_t[:],
                        psums[mb][:],
                        bias_sb[:, nb * N_TILE : (nb + 1) * N_TILE],
                    )
                    oeng = nc.gpsimd if mb % 2 == 0 else nc.scalar
                    oeng.dma_start(y3[:, mb, nb * N_TILE : (nb + 1) * N_TILE], o_t[:])

    nc.compile()
    return nc


def _get_nc():
    global _nc_cache
    if _nc_cache is None:
        _nc_cache = _build()
    return _nc_cache


def _make_in_maps(inputs, weights, bias):
    x = np.ascontiguousarray(np.asarray(inputs, dtype=np.float32))
    w = np.ascontiguousarray(np.asarray(weights, dtype=np.float32))
    b = np.asarray(bias, dtype=np.float32)

    xT = x.T  # [K, N_FULL] view
    wT = np.ascontiguousarray(w.T)  # [K, O]
    br = np.ascontiguousarray(np.broadcast_to(b[None, :], (P, O_DIM)))

    in_maps = []
    for c in range(N_CORES):
        xTc = np.ascontiguousarray(xT[:, c * M : (c + 1) * M])
        in_maps.append({"xT": xTc, "wT": wT, "biasr": br})
    return in_maps


def kernel(**inputs):
    nc = _get_nc()
    in_maps = _make_in_maps(inputs["inputs"], inputs["weights"], inputs["bias"])
    res = run_bass_kernel_spmd(nc, in_maps, core_ids=list(range(N_CORES)))
    return np.concatenate([r["y"] for r in res.results], axis=0)


def run_traced(inputs, weights, bias, **trace_kwargs):
    """Used by test.py: same computation, returns (output, BassKernelResults)."""
    nc = _get_nc()
    in_maps = _make_in_maps(inputs, weights, bias)
    res = run_bass_kernel_spmd(
        nc, in_maps, core_ids=list(range(N_CORES)), trace=True, **trace_kwargs
    )
    out = np.concatenate([r["y"] for r in res.results], axis=0)
    return out, res


# revision 10
# speedup vs baseline: 1.0653x; 1.0653x over previous
"""Trainium2 Bass kernel for y = inputs @ weights.T + bias.

Shapes: inputs [8192, 4096] f32, weights [4096, 4096] f32, bias [4096] f32,
output [8192, 4096] f32.

Strategy:
- Data-parallel across 8 NeuronCores: each core computes 1024 rows of the
  output; weights/bias are replicated.
- Host pre-transposes inputs and weights to K-major layout so the device
  kernel needs no on-chip transposes (fp32 has no DMA-transpose path).
- Matmuls run in float32r (TF32-like, full PE rate at moving-dim >= 256,
  ~1.5e-4 rel err at K=4096) accumulating fp32 in PSUM.
- Per core: cache the x-slice KxM [4096,1024] in SBUF (16.8 MB), stream W
  in [128,512] tiles, 8 PSUM banks accumulate the 8 row-blocks over K,
  bias added on DVE during PSUM eviction.
"""

import numpy as np

import concourse.bacc as bacc
import concourse.mybir as mybir
import concourse.tile as tile
from concourse.bass_utils import run_bass_kernel_spmd

N_CORES = 8
N_FULL = 8192  # input rows
K_DIM = 4096  # contraction (in features)
O_DIM = 4096  # out features
M = N_FULL // N_CORES  # rows per core (1024)
P = 128
KO = K_DIM // P  # 32 k-tiles
N_TILE = 512  # moving free dim per matmul (1 PSUM bank of fp32)
N_BLOCKS = O_DIM // N_TILE  # 8
M_BLOCKS = M // P  # 8

_nc_cache = None


def _build():
    nc = bacc.Bacc(target_bir_lowering=False)

    xT = nc.dram_tensor("xT", [K_DIM, M], mybir.dt.float32r, kind="ExternalInput")
    wT = nc.dram_tensor("wT", [K_DIM, O_DIM], mybir.dt.float32r, kind="ExternalInput")
    biasr = nc.dram_tensor("biasr", [P, O_DIM], mybir.dt.float32, kind="ExternalInput")
    y = nc.dram_tensor("y", [M, O_DIM], mybir.dt.float32, kind="ExternalOutput")

    xT3 = xT.ap().rearrange("(ko p) m -> p ko m", p=P)
    wT3 = wT.ap().rearrange("(ko p) n -> p ko n", p=P)
    y3 = y.ap().rearrange("(mb p) n -> p mb n", p=P)

    with tile.TileContext(nc) as tc:
        with (
            tc.tile_pool(name="persist", bufs=1) as persist,
            tc.tile_pool(name="wpool", bufs=10) as wpool,
            tc.tile_pool(name="opool", bufs=10) as opool,
            tc.tile_pool(name="psum", bufs=1, space="PSUM") as psum_pool,
        ):
            # Bias first (tiny, and the first PSUM drain gates on it).
            bias_sb = persist.tile([P, O_DIM], mybir.dt.float32, tag="bias")
            nc.sync.dma_start(bias_sb[:], biasr.ap())

            # x cached in SBUF, one tile per k-slab so matmuls can start as
            # soon as their slab has landed; loads split across two queues.
            x_sb = []
            for ko in range(KO):
                x_t = persist.tile([P, M], mybir.dt.float32r, tag=f"x{ko}")
                nc.gpsimd.dma_start(x_t[:], xT3[:, ko])
                x_sb.append(x_t)

            for nb in range(N_BLOCKS):
                psums = [
                    psum_pool.tile(
                        [P, N_TILE], mybir.dt.float32, tag=f"ps{m}", name=f"ps{m}"
                    )
                    for m in range(M_BLOCKS)
                ]
                for ko in range(KO):
                    w_t = wpool.tile([P, N_TILE], mybir.dt.float32r, tag="w")
                    weng = nc.sync if ko % 2 == 0 else nc.scalar
                    weng.dma_start(
                        w_t[:], wT3[:, ko, nb * N_TILE : (nb + 1) * N_TILE]
                    )
                    for mb in range(M_BLOCKS):
                        nc.tensor.matmul(
                            psums[mb][:],
                            x_sb[ko][:, mb * P : (mb + 1) * P],
                            w_t[:],
                            start=(ko == 0),
                            stop=(ko == KO - 1),
                        )
                for mb in range(M_BLOCKS):
                    o_t = opool.tile([P, N_TILE], mybir.dt.float32, tag="o")
                    nc.any.tensor_add(
                        o_t[:],
                        psums[mb][:],
                        bias_sb[:, nb * N_TILE : (nb + 1) * N_TILE],
                    )
                    oeng = nc.gpsimd if mb % 2 == 0 else nc.scalar
                    oeng.dma_start(y3[:, mb, nb * N_TILE : (nb + 1) * N_TILE], o_t[:])

    nc.compile()
    return nc


def _get_nc():
    global _nc_cache
    if _nc_cache is None:
        _nc_cache = _build()
    return _nc_cache


def _make_in_maps(inputs, weights, bias):
    x = np.ascontiguousarray(np.asarray(inputs, dtype=np.float32))
    w = np.ascontiguousarray(np.asarray(weights, dtype=np.float32))
    b = np.asarray(bias, dtype=np.float32)

    xT = x.T  # [K, N_FULL] view
    wT = np.ascontiguousarray(w.T)  # [K, O]
    br = np.ascontiguousarray(np.broadcast_to(b[None, :], (P, O_DIM)))

    in_maps = []
    for c in range(N_CORES):
        xTc = np.ascontiguousarray(xT[:, c * M : (c + 1) * M])
        in_maps.append({"xT": xTc, "wT": wT, "biasr": br})
    return in_maps


def kernel(**inputs):
    nc = _get_nc()
    in_maps = _make_in_maps(inputs["inputs"], inputs["weights"], inputs["bias"])
    res = run_bass_kernel_spmd(nc, in_maps, core_ids=list(range(N_CORES)))
    return np.concatenate([r["y"] for r in res.results], axis=0)


def run_traced(inputs, weights, bias, **trace_kwargs):
    """Used by test.py: same computation, returns (output, BassKernelResults)."""
    nc = _get_nc()
    in_maps = _make_in_maps(inputs, weights, bias)
    res = run_bass_kernel_spmd(
        nc, in_maps, core_ids=list(range(N_CORES)), trace=True, **trace_kwargs
    )
    out = np.concatenate([r["y"] for r in res.results], axis=0)
    return out, res


# revision 12
# speedup vs baseline: 1.0894x; 1.0226x over previous
"""Trainium2 Bass kernel for y = inputs @ weights.T + bias.

Shapes: inputs [8192, 4096] f32, weights [4096, 4096] f32, bias [4096] f32,
output [8192, 4096] f32.

Strategy:
- Data-parallel across 8 NeuronCores: each core computes 1024 rows of the
  output; weights/bias are replicated.
- Host pre-transposes inputs and weights to K-major layout so the device
  kernel needs no on-chip transposes (fp32 has no DMA-transpose path).
- Matmuls run in float32r (TF32-like, full PE rate at moving-dim >= 256,
  ~1.5e-4 rel err at K=4096) accumulating fp32 in PSUM.
- Per core: cache the x-slice KxM [4096,1024] in SBUF (16.8 MB), stream W
  in [128,512] tiles, 8 PSUM banks accumulate the 8 row-blocks over K,
  bias added on DVE during PSUM eviction.
"""

import numpy as np

import concourse.bacc as bacc
import concourse.mybir as mybir
import concourse.tile as tile
from concourse.bass_utils import run_bass_kernel_spmd

N_CORES = 8
N_FULL = 8192  # input rows
K_DIM = 4096  # contraction (in features)
O_DIM = 4096  # out features
M = N_FULL // N_CORES  # rows per core (1024)
P = 128
KO = K_DIM // P  # 32 k-tiles
N_TILE = 512  # moving free dim per matmul (1 PSUM bank of fp32)
N_BLOCKS = O_DIM // N_TILE  # 8
M_BLOCKS = M // P  # 8

_nc_cache = None


def _build():
    nc = bacc.Bacc(target_bir_lowering=False)

    xT = nc.dram_tensor("xT", [K_DIM, M], mybir.dt.float32r, kind="ExternalInput")
    wT = nc.dram_tensor("wT", [K_DIM, O_DIM], mybir.dt.float32r, kind="ExternalInput")
    biasr = nc.dram_tensor("biasr", [P, O_DIM], mybir.dt.float32, kind="ExternalInput")
    y = nc.dram_tensor("y", [M, O_DIM], mybir.dt.float32, kind="ExternalOutput")

    xT3 = xT.ap().rearrange("(ko p) m -> p ko m", p=P)
    wT3 = wT.ap().rearrange("(ko p) n -> p ko n", p=P)
    y3 = y.ap().rearrange("(mb p) n -> p mb n", p=P)

    with tile.TileContext(nc) as tc:
        with (
            tc.tile_pool(name="persist", bufs=1) as persist,
            tc.tile_pool(name="wpool", bufs=10) as wpool,
            tc.tile_pool(name="opool", bufs=10) as opool,
            tc.tile_pool(name="psum", bufs=1, space="PSUM") as psum_pool,
        ):
            # x cached in SBUF, one tile per k-slab so matmuls can start as
            # soon as their slab has landed. Bias comes in per-n-block chunks:
            # chunk 0 early (the first PSUM drain gates on it), the rest after
            # x is done - all on the gpsimd queue, off the w critical path.
            x_sb = []
            bias_sb = [None] * N_BLOCKS
            for ko in range(KO):
                x_t = persist.tile([P, M], mybir.dt.float32r, tag=f"x{ko}")
                nc.gpsimd.dma_start(x_t[:], xT3[:, ko])
                x_sb.append(x_t)
                if ko == 0:
                    b_t = persist.tile([P, N_TILE], mybir.dt.float32, tag="bias0")
                    nc.gpsimd.dma_start(b_t[:], biasr.ap()[:, :N_TILE])
                    bias_sb[0] = b_t
            for nb in range(1, N_BLOCKS):
                b_t = persist.tile([P, N_TILE], mybir.dt.float32, tag=f"bias{nb}")
                nc.gpsimd.dma_start(
                    b_t[:], biasr.ap()[:, nb * N_TILE : (nb + 1) * N_TILE]
                )
                bias_sb[nb] = b_t

            for nb in range(N_BLOCKS):
                psums = [
                    psum_pool.tile(
                        [P, N_TILE], mybir.dt.float32, tag=f"ps{m}", name=f"ps{m}"
                    )
                    for m in range(M_BLOCKS)
                ]
                for ko in range(KO):
                    w_t = wpool.tile([P, N_TILE], mybir.dt.float32r, tag="w")
                    weng = nc.sync if ko % 2 == 0 else nc.scalar
                    weng.dma_start(
                        w_t[:], wT3[:, ko, nb * N_TILE : (nb + 1) * N_TILE]
                    )
                    for mb in range(M_BLOCKS):
                        nc.tensor.matmul(
                            psums[mb][:],
                            x_sb[ko][:, mb * P : (mb + 1) * P],
                            w_t[:],
                            start=(ko == 0),
                            stop=(ko == KO - 1),
                        )
                for mb in range(M_BLOCKS):
                    o_t = opool.tile([P, N_TILE], mybir.dt.float32, tag="o")
                    nc.any.tensor_add(o_t[:], psums[mb][:], bias_sb[nb][:])
                    if nb == N_BLOCKS - 1:
                        # w streams are done; use the idle sync/scalar queues
                        # so the tail flush isn't serialized behind gpsimd.
                        oeng = nc.sync if mb % 2 == 0 else nc.scalar
                    else:
                        oeng = nc.gpsimd if mb % 2 == 0 else nc.scalar
                    oeng.dma_start(y3[:, mb, nb * N_TILE : (nb + 1) * N_TILE], o_t[:])

    nc.compile()
    return nc


def _get_nc():
    global _nc_cache
    if _nc_cache is None:
        _nc_cache = _build()
    return _nc_cache


def _make_in_maps(inputs, weights, bias):
    x = np.ascontiguousarray(np.asarray(inputs, dtype=np.float32))
    w = np.ascontiguousarray(np.asarray(weights, dtype=np.float32))
    b = np.asarray(bias, dtype=np.float32)

    xT = x.T  # [K, N_FULL] view
    wT = np.ascontiguousarray(w.T)  # [K, O]
    br = np.ascontiguousarray(np.broadcast_to(b[None, :], (P, O_DIM)))

    in_maps = []
    for c in range(N_CORES):
        xTc = np.ascontiguousarray(xT[:, c * M : (c + 1) * M])
        in_maps.append({"xT": xTc, "wT": wT, "biasr": br})
    return in_maps


def kernel(**inputs):
    nc = _get_nc()
    in_maps = _make_in_maps(inputs["inputs"], inputs["weights"], inputs["bias"])
    res = run_bass_kernel_spmd(nc, in_maps, core_ids=list(range(N_CORES)))
    return np.concatenate([r["y"] for r in res.results], axis=0)


def run_traced(inputs, weights, bias, **trace_kwargs):
    """Used by test.py: same computation, returns (output, BassKernelResults)."""
    nc = _get_nc()
    in_maps = _make_in_maps(inputs, weights, bias)
    res = run_bass_kernel_spmd(
        nc, in_maps, core_ids=list(range(N_CORES)), trace=True, **trace_kwargs
    )
    out = np.concatenate([r["y"] for r in res.results], axis=0)
    return out, res
